# revision 29
# baseline (speedup 1.0000x reference)
"""GAT-style attention kernel for Trainium2, data-parallel over batch on 8 cores.

Math: the reference computes
    e[i,j]  = lr_row[i] + lr_col[j]            (rank-1 score structure)
    atten   = softmax_j(where(mask>0, e, -1e9))
    out     = atten @ (x @ Wx.T + bx)
Because lr_row[i] is constant along the softmax axis j, it cancels:
    atten[i,j] = mask[i,j] * w[j] / sum_j mask[i,j] * w[j],
    w[j] = exp(lr_col[j])        (|lr_col| <~ 3, so no max-shift needed)
and since attention rows sum to 1, the bias passes through, so with
xv = x @ Wx.T + bx:
    out = (M @ (w * xv)) / (M @ w)
So the whole kernel is one [N,N] x [N,129] matmul per batch, normalized
row-wise, with tiny setup.  Memory-bound on the int32 mask read (16MB/core).

Layout choice: the per-core mask slice and x slice are laid out TRANSPOSED in
DRAM (host-side np layout op; dtypes unchanged).  This lets PE consume mask
chunks directly as the stationary operand (contraction over j on partitions)
with zero on-chip transposes.

Per core (batch b), pipeline:
  - maskT strips [128 j, N i] int32 DMA on the sync HWDGE ring (nothing else
    queues there), x/params on the scalar ring
  - DVE does ONLY the int32->bf16 strip casts (plus PSUM memset + epilogue),
    so the cast pipeline never blocks on the setup chain
  - setup smalls (leaky-relu scoring chain, U weight scaling) run on
    GpSimd/ACT; bias bx is added via a rank-1 (K=1) matmul into the
    projection PSUM
  - PE: matmul(acc[ti], lhsT=maskT_bf16[:, ti*128:(ti+1)*128], rhs=U[tj])
    accumulates 16 persistent PSUM accumulators [128, 132] over 16 j-strips;
    U[tj][:, :128] = w*xv strip, U[tj][:, 128] = w strip
  - PSUM packing: 3 accumulators per 2KB bank (6 banks). All matmuls use
    start=False; the accumulator tile is DVE-memset to 0 up front, so the
    per-element has_written bit makes the first touch overwrite-0 or add-to-0
    (both correct) and the whole-bank clear of start=True never fires.
  - epilogue: phase-parallel (16 reciprocals on DVE; 16 normalize-copies split
    ACT/DVE; 4 batched strip stores split across both HWDGE rings)
"""

import os
import sys

import numpy as np

for _p in ("/opt/trn_rl_repo",):
    if _p not in sys.path and os.path.isdir(_p):
        sys.path.append(_p)

import concourse.bacc as bacc
import concourse.bass as bass
import concourse.bass_isa as bass_isa
import concourse.tile as tile
from concourse import mybir
from concourse.bass_utils import run_bass_kernel_spmd

B, N, DIN, DOUT, DA = 8, 2048, 128, 128, 2
NEG_SLOPE = 0.2
P = 128
UC = 132  # U free width: 128 numerator cols + 1 denom col + 3 pad
PSB = 512  # fp32 words per PSUM bank

F32 = mybir.dt.float32
BF16 = mybir.dt.bfloat16
I32 = mybir.dt.int32

# bf16 matmul-params column layout (packed [P, 388] bf16 tensor)
PC_W = 0      # 0:130   wcomb = [Wx.T | Wc.T]
PC_BX = 130   # 130:260 [bx | 0 0] (used as 1-partition row)
PC_ONE = 260  # 260:388 ones (used as 1-partition row)
PCOLS = 388


def build(n=N, mask_bufs=7, cast_bufs=10):
    """Build the single-core program (all 8 cores run it SPMD)."""
    nt = n // P
    nc = bacc.Bacc(
        "TRN2",
        target_bir_lowering=False,
        debug=False,
        enable_asserts=False,
        num_devices=1,
    )
    xT_d = nc.dram_tensor("xT", [DIN, n], F32, kind="ExternalInput").ap()
    m_d = nc.dram_tensor("maskT", [n, n], I32, kind="ExternalInput").ap()
    par_d = nc.dram_tensor("parb", [P, PCOLS], BF16, kind="ExternalInput").ap()
    pf_d = nc.dram_tensor(
        "pf32", [P, DA + 2 * (DOUT + DA)], F32, kind="ExternalInput"
    ).ap()
    out_d = nc.dram_tensor("out", [n, DOUT], F32, kind="ExternalOutput").ap()

    from contextlib import ExitStack

    with tile.TileContext(nc) as tc, ExitStack() as ctx:
        consts = ctx.enter_context(tc.tile_pool(name="consts", bufs=1))
        small = ctx.enter_context(tc.tile_pool(name="small", bufs=2))
        mpool = ctx.enter_context(tc.tile_pool(name="mpool", bufs=mask_bufs))
        cpool = ctx.enter_context(tc.tile_pool(name="cpool", bufs=cast_bufs))
        ps_small = ctx.enter_context(tc.tile_pool(name="ps_small", bufs=2, space="PSUM"))
        ps_acc = ctx.enter_context(tc.tile_pool(name="ps_acc", bufs=1, space="PSUM"))

        # ---- persistent PSUM accumulators: 16 x [P, UC] packed 3-per-bank ----
        pacc = ps_acc.tile([P, 6, PSB], F32)

        def acc(ti):
            # bank = ti % 6: consecutive strips live in different PSUM banks so
            # the epilogue's ACT/DVE reads never serialize on a shared bank
            b, s = ti % 6, ti // 6
            return pacc[:, b, s * UC : (s + 1) * UC]

        # ---- loads: params then xT, both on the scalar ring ----
        par = consts.tile([P, PCOLS], BF16)
        nc.scalar.dma_start(par[:], par_d)
        pf32 = consts.tile([P, DA + 2 * (DOUT + DA)], F32)
        nc.scalar.dma_start(pf32[:], pf_d)
        a2b = pf32[:, 0:DA]
        bxp = pf32[:, DA : DA + 2 * (DOUT + DA)].rearrange(
            "p (k c) -> p k c", k=2
        )
        # xT rides the scalar ring: slower, but keeps the sync ring free so
        # the mask stream starts at t=0 (MM start is no longer critical)
        xT = consts.tile([P, n], F32)
        nc.scalar.dma_start(xT[:], xT_d)
        # cast x to bf16 on ACT (keeps DVE free for the mask-cast pipeline)
        xTb = consts.tile([P, n], BF16)
        nc.scalar.copy(xTb[:], xT[:])

        # ---- projections: xvcol[n,130] = xT_chunk.T @ [WxT|WcT] + [bx|0] ----
        # two projection tiles share one PSUM bank (2x520B <= 2KB); the pair is
        # evacuated with a single DVE tensor-tensor op that fuses the +bx add
        xvcol = consts.tile([P, nt, DOUT + DA], F32)
        for t0 in range(0, nt, 2):
            pxv = ps_small.tile([P, 2, DOUT + DA], F32, tag="pxv")
            for k in range(2):
                t = t0 + k
                nc.tensor.matmul(
                    pxv[:, k], xTb[:, t * P : (t + 1) * P],
                    par[:, PC_W : PC_W + DOUT + DA],
                    start=True, stop=True,
                )
            nc.vector.tensor_add(xvcol[:, t0 : t0 + 2], pxv[:], bxp)

        # ---- scoring chain (DVE, tiny): w = exp(a2 . LeakyReLU(col)) ----
        colp = xvcol[:, :, DOUT : DOUT + DA]  # [P, nt, 2] strided view
        c02 = small.tile([P, nt, DA], F32)
        nc.vector.tensor_scalar_mul(c02[:], colp, NEG_SLOPE)
        clr = small.tile([P, nt, DA], F32)
        nc.vector.tensor_max(clr[:], colp, c02[:])
        lr0 = small.tile([P, nt], F32)
        nc.vector.tensor_scalar(
            lr0[:], clr[:, :, 0], a2b[:, 0:1], None, mybir.AluOpType.mult
        )
        lr1 = small.tile([P, nt], F32)
        nc.vector.tensor_scalar(
            lr1[:], clr[:, :, 1], a2b[:, 1:2], None, mybir.AluOpType.mult
        )
        lrc = small.tile([P, nt], F32)
        nc.vector.tensor_add(lrc[:], lr0[:], lr1[:])
        w_all = consts.tile([P, nt], F32)
        nc.scalar.activation(w_all[:], lrc[:], mybir.ActivationFunctionType.Exp)

        # ---- U chunks [P, nt, UC] bf16: U[:,:,0:128]=w*xv, U[:,:,128]=w ----
        U = consts.tile([P, nt, UC], BF16)
        nc.vector.memset(U[:, :, DOUT + 1 : UC], 0)
        for t in range(nt):
            if t % 2 == 0:
                nc.scalar.activation(
                    U[:, t, 0:DOUT],
                    xvcol[:, t, 0:DOUT],
                    mybir.ActivationFunctionType.Copy,
                    scale=w_all[:, t : t + 1],
                )
            else:
                nc.vector.tensor_scalar(
                    U[:, t, 0:DOUT], xvcol[:, t, 0:DOUT], w_all[:, t : t + 1], None,
                    mybir.AluOpType.mult,
                )
        nc.gpsimd.tensor_copy(U[:, :, DOUT], w_all[:])

        # ---- main loop over j-strips of maskT ----
        # strip 0's ti<6 matmuls are each the first touch of their bank:
        # start=True clears the whole bank's has_written bits before the
        # bank's other two slots are first written.
        # The last strip is loaded/cast in two column halves with independent
        # completion semaphores: the straggling SDMA engine's completion
        # increment lags the data by several us, and the halves let the final
        # cast+matmul block start earlier.
        nh = n // 2
        for tj in range(nt):
            last = tj == nt - 1
            mi32 = mpool.tile([P, n], I32)
            mbf = cpool.tile([P, n], BF16)
            for h in range(2 if last else 1):
                w0, w1 = (h * nh, (h + 1) * nh) if last else (0, n)
                nc.sync.dma_start(
                    mi32[:, w0:w1], m_d[tj * P : (tj + 1) * P, w0:w1]
                )
                nc.vector.tensor_copy(mbf[:, w0:w1], mi32[:, w0:w1])
                for ti in range(w0 // P, w1 // P):
                    nc.tensor.matmul(
                        acc(ti),
                        mbf[:, ti * P : (ti + 1) * P],
                        U[:, tj],
                        start=(tj == 0 and ti < 6),
                        stop=last,
                    )

        # ---- epilogue ----
        # The tile framework serializes concurrent ACT/DVE PSUM accesses, so
        # only ACT touches PSUM here (6 wide bank copies); DVE computes the
        # reciprocals and most scales from SBUF, where ACT and DVE overlap.
        obuf = consts.tile([P, nt, DOUT], F32)
        sraw = consts.tile([P, 6, 3 * UC], F32)
        recs = consts.tile([P, 6, 3], F32)
        out_r = out_d.rearrange("(t p) c -> p t c", p=P)
        # emit every PSUM evacuation copy before any rec/scale so the ACT
        # FIFO drains the copies back-to-back instead of stalling on scales.
        # Two wide copies: banks 0-3 (one strided op) then banks 4-5.
        nc.scalar.copy(sraw[:, 0:4, 0 : 3 * UC], pacc[:, 0:4, 0 : 3 * UC])
        nc.scalar.copy(sraw[:, 4:6, 0 : 2 * UC], pacc[:, 4:6, 0 : 2 * UC])
        for b in range(6):
            ns = 3 if b < 4 else 2  # banks 4,5 hold only 2 accumulators
            nc.vector.reciprocal(
                recs[:, b, 0:ns].rearrange("p (s u) -> p s u", u=1),
                sraw[:, b, 0 : ns * UC].rearrange("p (s u) -> p s u", u=UC)[
                    :, :, DOUT : DOUT + 1
                ],
            )
        # normalize-scales split 6 ACT / 10 DVE; all store issues ride the
        # idle sync ring so they never steal ACT engine time from the scales
        for ti in range(nt):
            b, s = ti % 6, ti // 6
            if ti % 3 == 0:
                nc.scalar.activation(
                    obuf[:, ti], sraw[:, b, s * UC : s * UC + DOUT],
                    mybir.ActivationFunctionType.Copy,
                    scale=recs[:, b, s : s + 1],
                )
            else:
                nc.vector.tensor_scalar(
                    obuf[:, ti], sraw[:, b, s * UC : s * UC + DOUT],
                    recs[:, b, s : s + 1], None, mybir.AluOpType.mult,
                )
            if ti % 4 == 3:
                g = ti // 4
                # last group rides the scalar ring: ACT's scales are done by
                # then, and it skips the sync ring's serialized issue queue
                q = nc.scalar if g == 3 else nc.sync
                q.dma_start(
                    out_r[:, g * 4 : (g + 1) * 4, :], obuf[:, g * 4 : (g + 1) * 4, :]
                )

    nc.compile()
    return nc


def host_inputs(x, mask, Wc, Wcat, Wx, bx, b):
    """Per-core input map for batch b (weights replicated; layout host-prepped,
    dtypes preserved: mask stays int32, x stays float32)."""
    import ml_dtypes

    par = np.zeros((P, PCOLS), dtype=ml_dtypes.bfloat16)
    par[:, PC_W : PC_W + DOUT + DA] = np.concatenate([Wx.T, Wc.T], axis=1).astype(
        ml_dtypes.bfloat16
    )
    par[:, PC_BX : PC_BX + DOUT] = bx.reshape(1, DOUT).astype(ml_dtypes.bfloat16)
    par[:, PC_ONE : PC_ONE + P] = 1.0
    pf = np.zeros((P, DA + 2 * (DOUT + DA)), dtype=np.float32)
    pf[:, 0:DA] = Wcat[DA:].reshape(1, DA)
    bxrow = np.zeros(DOUT + DA, dtype=np.float32)
    bxrow[0:DOUT] = bx
    pf[:, DA : DA + (DOUT + DA)] = bxrow
    pf[:, DA + (DOUT + DA) : DA + 2 * (DOUT + DA)] = bxrow
    return {
        "xT": np.ascontiguousarray(x[b].T, dtype=np.float32),
        "maskT": np.ascontiguousarray(mask[b].T),
        "parb": par,
        "pf32": pf,
    }


_cached = {}


def _get_nc():
    if "nc" not in _cached:
        _cached["nc"] = build()
    return _cached["nc"]


def _install_ntff_shim():
    """The agent image's antenv lacks axon_hooks; synthesize it so
    run_bass_kernel_spmd(trace=True) can reach the .so's NTFF profiler."""
    import types

    try:
        import antenv.axon_hooks  # noqa: F401

        return True
    except ImportError:
        pass
    try:
        import antenv
        from trn_agent_boot.trn_boot import _ntff_profile_via_ctypes

        hook = _ntff_profile_via_ctypes("/opt/axon/libaxon_pjrt.so")
        mod = types.ModuleType("antenv.axon_hooks")
        _state = {"hook": hook}
        mod.set_axon_ntff_profile_hook = lambda h: _state.__setitem__("hook", h)
        mod.get_axon_ntff_profile_hook = lambda: _state["hook"]
        sys.modules["antenv.axon_hooks"] = mod
        antenv.axon_hooks = mod
        return hook is not None
    except Exception as e:
        print(f"ntff shim failed: {e}", file=sys.stderr)
        return False


def kernel(x, mask, Wr, Wc, Wcat, Wx, bx, _trace=False, **_unused):
    x = np.asarray(x)
    mask = np.asarray(mask)
    Wc = np.asarray(Wc)
    Wcat = np.asarray(Wcat)
    Wx = np.asarray(Wx)
    bx = np.asarray(bx)
    nc = _get_nc()
    if _trace:
        _trace = _install_ntff_shim()
    in_maps = [host_inputs(x, mask, Wc, Wcat, Wx, bx, b) for b in range(B)]
    res = run_bass_kernel_spmd(nc, in_maps, core_ids=list(range(B)), trace=_trace)
    out = np.stack([res.results[c]["out"] for c in range(B)]).astype(np.float32)
    if _trace:
        kernel.last_results = res
    return out


# revision 30
# speedup vs baseline: 1.0242x; 1.0242x over previous
"""GAT-style attention kernel for Trainium2, data-parallel over batch on 8 cores.

Math: the reference computes
    e[i,j]  = lr_row[i] + lr_col[j]            (rank-1 score structure)
    atten   = softmax_j(where(mask>0, e, -1e9))
    out     = atten @ (x @ Wx.T + bx)
Because lr_row[i] is constant along the softmax axis j, it cancels:
    atten[i,j] = mask[i,j] * w[j] / sum_j mask[i,j] * w[j],
    w[j] = exp(lr_col[j])        (|lr_col| <~ 3, so no max-shift needed)
and since attention rows sum to 1, the bias passes through, so with
xv = x @ Wx.T + bx:
    out = (M @ (w * xv)) / (M @ w)
So the whole kernel is one [N,N] x [N,129] matmul per batch, normalized
row-wise, with tiny setup.  Memory-bound on the int32 mask read (16MB/core).

Layout choice: the per-core mask slice and x slice are laid out TRANSPOSED in
DRAM (host-side np layout op; dtypes unchanged).  This lets PE consume mask
chunks directly as the stationary operand (contraction over j on partitions)
with zero on-chip transposes.

Per core (batch b), pipeline:
  - maskT strips [128 j, N i] int32 DMA on the sync HWDGE ring (nothing else
    queues there), x/params on the scalar ring
  - DVE does ONLY the int32->bf16 strip casts (plus PSUM memset + epilogue),
    so the cast pipeline never blocks on the setup chain
  - setup smalls (leaky-relu scoring chain, U weight scaling) run on
    GpSimd/ACT; bias bx is added via a rank-1 (K=1) matmul into the
    projection PSUM
  - PE: matmul(acc[ti], lhsT=maskT_bf16[:, ti*128:(ti+1)*128], rhs=U[tj])
    accumulates 16 persistent PSUM accumulators [128, 132] over 16 j-strips;
    U[tj][:, :128] = w*xv strip, U[tj][:, 128] = w strip
  - PSUM packing: 3 accumulators per 2KB bank (6 banks). All matmuls use
    start=False; the accumulator tile is DVE-memset to 0 up front, so the
    per-element has_written bit makes the first touch overwrite-0 or add-to-0
    (both correct) and the whole-bank clear of start=True never fires.
  - epilogue: phase-parallel (16 reciprocals on DVE; 16 normalize-copies split
    ACT/DVE; 4 batched strip stores split across both HWDGE rings)
"""

import os
import sys

import numpy as np

for _p in ("/opt/trn_rl_repo",):
    if _p not in sys.path and os.path.isdir(_p):
        sys.path.append(_p)

import concourse.bacc as bacc
import concourse.bass as bass
import concourse.bass_isa as bass_isa
import concourse.tile as tile
from concourse import mybir
from concourse.bass_utils import run_bass_kernel_spmd

B, N, DIN, DOUT, DA = 8, 2048, 128, 128, 2
NEG_SLOPE = 0.2
P = 128
UC = 132  # U free width: 128 numerator cols + 1 denom col + 3 pad
PSB = 512  # fp32 words per PSUM bank

F32 = mybir.dt.float32
BF16 = mybir.dt.bfloat16
I32 = mybir.dt.int32

# bf16 matmul-params column layout (packed [P, 388] bf16 tensor)
PC_W = 0      # 0:130   wcomb = [Wx.T | Wc.T]
PC_BX = 130   # 130:260 [bx | 0 0] (used as 1-partition row)
PC_ONE = 260  # 260:388 ones (used as 1-partition row)
PCOLS = 388


def build(n=N, mask_bufs=7, cast_bufs=10):
    """Build the single-core program (all 8 cores run it SPMD)."""
    nt = n // P
    nc = bacc.Bacc(
        "TRN2",
        target_bir_lowering=False,
        debug=False,
        enable_asserts=False,
        num_devices=1,
    )
    xT_d = nc.dram_tensor("xT", [DIN, n], F32, kind="ExternalInput").ap()
    m_d = nc.dram_tensor("maskT", [n, n], I32, kind="ExternalInput").ap()
    par_d = nc.dram_tensor("parb", [P, PCOLS], BF16, kind="ExternalInput").ap()
    pf_d = nc.dram_tensor(
        "pf32", [P, DA + 2 * (DOUT + DA)], F32, kind="ExternalInput"
    ).ap()
    out_d = nc.dram_tensor("out", [n, DOUT], F32, kind="ExternalOutput").ap()

    from contextlib import ExitStack

    with tile.TileContext(nc) as tc, ExitStack() as ctx:
        consts = ctx.enter_context(tc.tile_pool(name="consts", bufs=1))
        small = ctx.enter_context(tc.tile_pool(name="small", bufs=2))
        mpool = ctx.enter_context(tc.tile_pool(name="mpool", bufs=mask_bufs))
        cpool = ctx.enter_context(tc.tile_pool(name="cpool", bufs=cast_bufs))
        ps_small = ctx.enter_context(tc.tile_pool(name="ps_small", bufs=2, space="PSUM"))
        ps_acc = ctx.enter_context(tc.tile_pool(name="ps_acc", bufs=1, space="PSUM"))

        # ---- persistent PSUM accumulators: 16 x [P, UC] packed 3-per-bank ----
        pacc = ps_acc.tile([P, 6, PSB], F32)

        def acc(ti):
            # bank = ti % 6: consecutive strips live in different PSUM banks so
            # the epilogue's ACT/DVE reads never serialize on a shared bank
            b, s = ti % 6, ti // 6
            return pacc[:, b, s * UC : (s + 1) * UC]

        # ---- loads: params then xT, both on the scalar ring ----
        par = consts.tile([P, PCOLS], BF16)
        nc.scalar.dma_start(par[:], par_d)
        pf32 = consts.tile([P, DA + 2 * (DOUT + DA)], F32)
        nc.scalar.dma_start(pf32[:], pf_d)
        a2b = pf32[:, 0:DA]
        bxp = pf32[:, DA : DA + 2 * (DOUT + DA)].rearrange(
            "p (k c) -> p k c", k=2
        )
        # xT rides the scalar ring: slower, but keeps the sync ring free so
        # the mask stream starts at t=0 (MM start is no longer critical)
        xT = consts.tile([P, n], F32)
        nc.scalar.dma_start(xT[:], xT_d)
        # cast x to bf16 on ACT (keeps DVE free for the mask-cast pipeline)
        xTb = consts.tile([P, n], BF16)
        nc.scalar.copy(xTb[:], xT[:])

        # ---- projections: xvcol[n,130] = xT_chunk.T @ [WxT|WcT] + [bx|0] ----
        # two projection tiles share one PSUM bank (2x520B <= 2KB); the pair is
        # evacuated with a single DVE tensor-tensor op that fuses the +bx add
        xvcol = consts.tile([P, nt, DOUT + DA], F32)
        for t0 in range(0, nt, 2):
            pxv = ps_small.tile([P, 2, DOUT + DA], F32, tag="pxv")
            for k in range(2):
                t = t0 + k
                nc.tensor.matmul(
                    pxv[:, k], xTb[:, t * P : (t + 1) * P],
                    par[:, PC_W : PC_W + DOUT + DA],
                    start=True, stop=True,
                )
            nc.vector.tensor_add(xvcol[:, t0 : t0 + 2], pxv[:], bxp)

        # ---- scoring chain (DVE, tiny): w = exp(a2 . LeakyReLU(col)) ----
        colp = xvcol[:, :, DOUT : DOUT + DA]  # [P, nt, 2] strided view
        c02 = small.tile([P, nt, DA], F32)
        nc.vector.tensor_scalar_mul(c02[:], colp, NEG_SLOPE)
        clr = small.tile([P, nt, DA], F32)
        nc.vector.tensor_max(clr[:], colp, c02[:])
        lr0 = small.tile([P, nt], F32)
        nc.vector.tensor_scalar(
            lr0[:], clr[:, :, 0], a2b[:, 0:1], None, mybir.AluOpType.mult
        )
        lr1 = small.tile([P, nt], F32)
        nc.vector.tensor_scalar(
            lr1[:], clr[:, :, 1], a2b[:, 1:2], None, mybir.AluOpType.mult
        )
        lrc = small.tile([P, nt], F32)
        nc.vector.tensor_add(lrc[:], lr0[:], lr1[:])
        w_all = consts.tile([P, nt], F32)
        nc.scalar.activation(w_all[:], lrc[:], mybir.ActivationFunctionType.Exp)

        # ---- U chunks [P, nt, UC] bf16: U[:,:,0:128]=w*xv, U[:,:,128]=w ----
        U = consts.tile([P, nt, UC], BF16)
        nc.vector.memset(U[:, :, DOUT + 1 : UC], 0)
        for t in range(nt):
            if t % 2 == 0:
                nc.scalar.activation(
                    U[:, t, 0:DOUT],
                    xvcol[:, t, 0:DOUT],
                    mybir.ActivationFunctionType.Copy,
                    scale=w_all[:, t : t + 1],
                )
            else:
                nc.vector.tensor_scalar(
                    U[:, t, 0:DOUT], xvcol[:, t, 0:DOUT], w_all[:, t : t + 1], None,
                    mybir.AluOpType.mult,
                )
        nc.gpsimd.tensor_copy(U[:, :, DOUT], w_all[:])

        # ---- main loop over j-strips of maskT ----
        # strip 0's ti<6 matmuls are each the first touch of their bank:
        # start=True clears the whole bank's has_written bits before the
        # bank's other two slots are first written.
        # The last strip is loaded/cast in two column halves with independent
        # completion semaphores: the straggling SDMA engine's completion
        # increment lags the data by several us, and the halves let the final
        # cast+matmul block start earlier.
        nh = n // 2
        for tj in range(nt):
            last = tj == nt - 1
            mi32 = mpool.tile([P, n], I32)
            mbf = cpool.tile([P, n], BF16)
            for h in range(2 if last else 1):
                w0, w1 = (h * nh, (h + 1) * nh) if last else (0, n)
                nc.sync.dma_start(
                    mi32[:, w0:w1], m_d[tj * P : (tj + 1) * P, w0:w1]
                )
                nc.vector.tensor_copy(mbf[:, w0:w1], mi32[:, w0:w1])
                for ti in range(w0 // P, w1 // P):
                    nc.tensor.matmul(
                        acc(ti),
                        mbf[:, ti * P : (ti + 1) * P],
                        U[:, tj],
                        start=(tj == 0 and ti < 6),
                        stop=last,
                    )

        # ---- epilogue ----
        # The tile framework serializes concurrent ACT/DVE PSUM accesses, so
        # only ACT touches PSUM here (6 wide bank copies); DVE computes the
        # reciprocals and most scales from SBUF, where ACT and DVE overlap.
        obuf = consts.tile([P, nt, DOUT], F32)
        sraw = consts.tile([P, 6, 3 * UC], F32)
        recs = consts.tile([P, 6, 3], F32)
        out_r = out_d.rearrange("(t p) c -> p t c", p=P)
        # emit every PSUM evacuation copy before any rec/scale so the ACT
        # FIFO drains the copies back-to-back instead of stalling on scales.
        # Two wide copies: banks 0-3 (one strided op) then banks 4-5.
        nc.scalar.copy(sraw[:, 4:6, 0 : 2 * UC], pacc[:, 4:6, 0 : 2 * UC])
        nc.scalar.copy(sraw[:, 0:4, 0 : 3 * UC], pacc[:, 0:4, 0 : 3 * UC])
        for b in (4, 5, 0, 1, 2, 3):
            ns = 3 if b < 4 else 2  # banks 4,5 hold only 2 accumulators
            nc.vector.reciprocal(
                recs[:, b, 0:ns].rearrange("p (s u) -> p s u", u=1),
                sraw[:, b, 0 : ns * UC].rearrange("p (s u) -> p s u", u=UC)[
                    :, :, DOUT : DOUT + 1
                ],
            )
        # normalize-scales split 6 ACT / 10 DVE; all store issues ride the
        # idle sync ring so they never steal ACT engine time from the scales
        for ti in range(nt):
            b, s = ti % 6, ti // 6
            if ti % 4 == 1:
                nc.scalar.activation(
                    obuf[:, ti], sraw[:, b, s * UC : s * UC + DOUT],
                    mybir.ActivationFunctionType.Copy,
                    scale=recs[:, b, s : s + 1],
                )
            else:
                nc.vector.tensor_scalar(
                    obuf[:, ti], sraw[:, b, s * UC : s * UC + DOUT],
                    recs[:, b, s : s + 1], None, mybir.AluOpType.mult,
                )
            if ti % 4 == 3:
                g = ti // 4
                # last group rides the scalar ring: ACT's scales are done by
                # then, and it skips the sync ring's serialized issue queue
                q = nc.scalar if g == 3 else nc.sync
                q.dma_start(
                    out_r[:, g * 4 : (g + 1) * 4, :], obuf[:, g * 4 : (g + 1) * 4, :]
                )

    nc.compile()
    return nc


def host_inputs(x, mask, Wc, Wcat, Wx, bx, b):
    """Per-core input map for batch b (weights replicated; layout host-prepped,
    dtypes preserved: mask stays int32, x stays float32)."""
    import ml_dtypes

    par = np.zeros((P, PCOLS), dtype=ml_dtypes.bfloat16)
    par[:, PC_W : PC_W + DOUT + DA] = np.concatenate([Wx.T, Wc.T], axis=1).astype(
        ml_dtypes.bfloat16
    )
    par[:, PC_BX : PC_BX + DOUT] = bx.reshape(1, DOUT).astype(ml_dtypes.bfloat16)
    par[:, PC_ONE : PC_ONE + P] = 1.0
    pf = np.zeros((P, DA + 2 * (DOUT + DA)), dtype=np.float32)
    pf[:, 0:DA] = Wcat[DA:].reshape(1, DA)
    bxrow = np.zeros(DOUT + DA, dtype=np.float32)
    bxrow[0:DOUT] = bx
    pf[:, DA : DA + (DOUT + DA)] = bxrow
    pf[:, DA + (DOUT + DA) : DA + 2 * (DOUT + DA)] = bxrow
    return {
        "xT": np.ascontiguousarray(x[b].T, dtype=np.float32),
        "maskT": np.ascontiguousarray(mask[b].T),
        "parb": par,
        "pf32": pf,
    }


_cached = {}


def _get_nc():
    if "nc" not in _cached:
        _cached["nc"] = build()
    return _cached["nc"]


def _install_ntff_shim():
    """The agent image's antenv lacks axon_hooks; synthesize it so
    run_bass_kernel_spmd(trace=True) can reach the .so's NTFF profiler."""
    import types

    try:
        import antenv.axon_hooks  # noqa: F401

        return True
    except ImportError:
        pass
    try:
        import antenv
        from trn_agent_boot.trn_boot import _ntff_profile_via_ctypes

        hook = _ntff_profile_via_ctypes("/opt/axon/libaxon_pjrt.so")
        mod = types.ModuleType("antenv.axon_hooks")
        _state = {"hook": hook}
        mod.set_axon_ntff_profile_hook = lambda h: _state.__setitem__("hook", h)
        mod.get_axon_ntff_profile_hook = lambda: _state["hook"]
        sys.modules["antenv.axon_hooks"] = mod
        antenv.axon_hooks = mod
        return hook is not None
    except Exception as e:
        print(f"ntff shim failed: {e}", file=sys.stderr)
        return False


def kernel(x, mask, Wr, Wc, Wcat, Wx, bx, _trace=False, **_unused):
    x = np.asarray(x)
    mask = np.asarray(mask)
    Wc = np.asarray(Wc)
    Wcat = np.asarray(Wcat)
    Wx = np.asarray(Wx)
    bx = np.asarray(bx)
    nc = _get_nc()
    if _trace:
        _trace = _install_ntff_shim()
    in_maps = [host_inputs(x, mask, Wc, Wcat, Wx, bx, b) for b in range(B)]
    res = run_bass_kernel_spmd(nc, in_maps, core_ids=list(range(B)), trace=_trace)
    out = np.stack([res.results[c]["out"] for c in range(B)]).astype(np.float32)
    if _trace:
        kernel.last_results = res
    return out
